# revision 11
# baseline (speedup 1.0000x reference)
"""LRU (Linear Recurrent Unit) block kernel for Trainium2, 8 NeuronCores.

Math (per batch element, see reference):
    lam  = exp(-exp(nu_log)) * exp(i*exp(theta_log))          (S,) complex
    Bn   = (B_re + i B_im) * exp(gamma_log)[:, None]          (S, D)
    Bu_t = Bn @ x_t                                           complex
    s_t  = lam * s_{t-1} + Bu_t                               diagonal complex scan
    z_t  = Re(C s_t) + D x_t
    out  = W_proj @ gelu(W_fc @ z + b_fc) + b_proj + x        (MLP + residual)

Device strategy (data-parallel over batch, 2 sequences/core):
  - Everything runs transposed: features on SBUF partitions, tokens on the
    free axis. x is pre-transposed on the host (fp32 DMA-transpose is
    unsupported; PE transposes would waste cycles).
  - Complex scan via the modulus-phase decomposition: with lam = r*e^{i*th},
    v_tau = e^{-i*th*tau} s_tau obeys v_tau = r v_{tau-1} + e^{-i*th*tau} Bu_tau
    — TWO REAL first-order recurrences that map onto the DVE
    tensor_tensor_scan instruction (fp32 internal state, fp32 decay).
  - The twiddle (cos/sin modulation) runs in fp16 on the DVE 2x mode; tables
    are host-precomputed in float64. Chunk carries rotate by e^{i*th*SC}.
  - Matmuls: float32r (full-speed PE) for B/D/MLP; fp16 for the C projection
    (its rhs comes from the fp16 twiddle path).
  - PSUM-sized sub-chunks of 256 positions for matmuls; DVE/scan work runs on
    1024-position super-chunks to amortize instruction overhead.
"""

import numpy as np

import concourse.bass as bass
import concourse.mybir as mybir
import concourse.tile as tile
from concourse.vector_clock import ScopedClock
from concourse.bass_utils import run_bass_kernel_spmd

Alu = mybir.AluOpType
F32 = mybir.dt.float32
F32R = mybir.dt.float32r
F16 = mybir.dt.float16
ACTF = mybir.ActivationFunctionType
GELU_FUNC = ACTF.Gelu  # overridable for CoreSim (no Gelu in the interpreter)

BATCH, SEQLEN, DM, DS, DF = 16, 8192, 256, 256, 1024
NCORES = 8
NSEQ = BATCH // NCORES          # sequences per core
PC = 256                        # positions per PSUM sub-chunk (per sequence)
SC = 1024                       # positions per DVE super-chunk

# ---- f32r consts blob layout (columns of [128, NCOL]) ----------------------
# 45 weight tiles: bnre 4, bnim 4, dT 4, wfc 16, wpj 16, identity 1
def _wi_bnre(kt, st): return 0 + kt * 2 + st
def _wi_bnim(kt, st): return 4 + kt * 2 + st
def _wi_dT(kt, ot):   return 8 + kt * 2 + ot
def _wi_wfc(kt, ft):  return 12 + kt * 8 + ft
def _wi_wpj(ft, ot):  return 28 + ft * 2 + ot
_WI_IDENT = 44
NW = 45
RT0 = NW * 128                  # scan decay r, [st][tau]: 2*SC fp32 cols
ROT0 = RT0 + 2 * SC             # carry rotation [rotc0, rotc1, rots0, rots1]
BFC0 = ROT0 + 4                 # fc1 bias per f-tile (8)
BPJ0 = BFC0 + 8                 # proj bias per o-tile (2)
NCOL = BPJ0 + 2

# ---- fp16 consts blob layout (columns of [128, NCOLH]) ---------------------
def _hi_cr(st, ot): return st * 2 + ot      # C_re^T tiles
def _hi_cm(st, ot): return 4 + st * 2 + ot  # -C_im^T tiles
CH0 = 8 * 128                   # cos table [st][tau]: 2*SC cols
SH0 = CH0 + 2 * SC              # sin table
NCOLH = SH0 + 2 * SC


# --- tile-exit drain workaround: walrus in this container caps the sync-wait
# slots on a TPB_CTRL Drain; split the waits onto follow-up SP nops. ---------
def _patched_drain_and_barrier(self, tick_clock, wait_clock):
    nc = self.nc
    drain_inst = nc.sync.drain()
    wait_clock.add_sem_waits(
        drain_inst.ins, ScopedClock({None: tick_clock.global_clock})
    )
    si = drain_inst.ins.sync_info
    if si is not None and si.on_wait and len(si.on_wait) > 1:
        waits = list(si.on_wait)
        drain_inst.ins.sync_info = mybir.SyncInfo(
            on_wait=[waits[0]], on_update=list(si.on_update or [])
        )
        for w in waits[1:]:
            nop = nc.sync.nop(hint="drain_wait_split", nofuse=True)
            nop.ins.sync_info = mybir.SyncInfo(on_wait=[w], on_update=[])
    nc.all_engine_barrier()
    assert self.sems is not None
    popped = nc._tile_sem_poison_stack.pop()
    assert popped is self._sem_poison
    nc.clear_and_free_semaphores(list(self.sems.allocated().values()))
    nc.all_engine_barrier()


tile.TileContext._drain_and_barrier = _patched_drain_and_barrier


# --- universal sync-wait splitter: this walrus rejects >1 wait on several
# instruction structs (S3_LW matmul, TPB_CTRL drain, ...). Rewrite the
# serialized BIR so every instruction carries at most one wait; extras move
# to injected same-engine NoOps placed immediately before it. ----------------
def _split_sync_waits(bir: bytes) -> bytes:
    import json as _json

    m = _json.loads(bir)
    ctr = 0
    for f in m.get("functions", []):
        for bb in f.get("blocks", []):
            insts = bb.get("instructions")
            if not insts:
                continue
            out = []
            for inst in insts:
                si = inst.get("sync_info")
                ow = (si or {}).get("on_wait") or []
                if len(ow) > 1:
                    for wdesc in ow[:-1]:
                        ctr += 1
                        out.append({
                            "engine": inst["engine"],
                            "ins": [],
                            "outs": [],
                            "name": f"I-wsplit{ctr}",
                            "opcode": "NoOp",
                            "sync_info": {"on_update": [], "on_wait": [wdesc]},
                            "text_hint": "wait_split",
                        })
                    si["on_wait"] = [ow[-1]]
                out.append(inst)
            bb["instructions"] = out
    return _json.dumps(m).encode()


_orig_to_json_bytes = bass.Bass.to_json_bytes


def _to_json_bytes_split(self):
    return _split_sync_waits(_orig_to_json_bytes(self))


bass.Bass.to_json_bytes = _to_json_bytes_split


def _enable_axon_ntff_profiling():
    """Best-effort: register the axon NTFF profile hook (the image's antenv
    lacks axon_hooks; the backing ctypes impl ships in trn_agent_boot) and
    neuter the S3 artifact upload the trace path would attempt."""
    try:
        import sys, types
        try:
            import antenv.axon_hooks  # noqa: F401
        except ImportError:
            mod = types.ModuleType("antenv.axon_hooks")
            mod._hook = None

            def set_axon_ntff_profile_hook(h):
                mod._hook = h

            def get_axon_ntff_profile_hook():
                return mod._hook

            mod.set_axon_ntff_profile_hook = set_axon_ntff_profile_hook
            mod.get_axon_ntff_profile_hook = get_axon_ntff_profile_hook
            sys.modules["antenv.axon_hooks"] = mod
            import antenv
            antenv.axon_hooks = mod
        import antenv.axon_hooks as ah
        if ah.get_axon_ntff_profile_hook() is None:
            from trn_agent_boot.trn_boot import _ntff_profile_via_ctypes
            ah.set_axon_ntff_profile_hook(
                _ntff_profile_via_ctypes("/opt/axon/libaxon_pjrt.so")
            )
        import concourse.bass_utils as bu
        bu.upload_artifacts = lambda tmpdir: ""
    except Exception:
        pass


import os as _os
if _os.environ.get("BASS_TRACE"):
    _enable_axon_ntff_profiling()


def build_nc(seqlen=SEQLEN, nseq=NSEQ, pc=PC, sc=SC):
    """Per-core Bass module. Token layout: [nseq, seqlen] flattened."""
    ntok = nseq * seqlen
    nsup = seqlen // sc
    subs = sc // pc
    assert seqlen % sc == 0 and sc % pc == 0

    nc = bass.Bass()
    xT = nc.declare_dram_parameter("xT", [2, 128, ntok], F32R, isOutput=False)
    consts = nc.declare_dram_parameter("consts", [128, NCOL], F32R, isOutput=False)
    constsh = nc.declare_dram_parameter("constsh", [128, NCOLH], F16, isOutput=False)
    outT = nc.declare_dram_parameter("outT", [2, 128, ntok], F32, isOutput=True)

    xTv = [xT[kt].rearrange("p (b l) -> p b l", b=nseq) for kt in range(2)]
    outTv = [outT[ot].rearrange("p (b l) -> p b l", b=nseq) for ot in range(2)]

    from contextlib import ExitStack
    with tile.TileContext(nc) as tc, ExitStack() as ctx:
        singles = ctx.enter_context(tc.tile_pool(name="singles", bufs=1))
        xts = ctx.enter_context(tc.tile_pool(name="xts", bufs=subs + 2))
        bus = ctx.enter_context(tc.tile_pool(name="bus", bufs=2))
        uts = ctx.enter_context(tc.tile_pool(name="uts", bufs=1))
        ss = ctx.enter_context(tc.tile_pool(name="ss", bufs=2))
        zs_p = ctx.enter_context(tc.tile_pool(name="zs", bufs=2))
        hs_p = ctx.enter_context(tc.tile_pool(name="hs", bufs=1))
        obs = ctx.enter_context(tc.tile_pool(name="obs", bufs=2))
        tmps = ctx.enter_context(tc.tile_pool(name="tmps", bufs=2))
        carries = ctx.enter_context(tc.tile_pool(name="carries", bufs=2))
        ps_bu = ctx.enter_context(tc.tile_pool(name="ps_bu", bufs=2, space="PSUM"))
        ps_z = ctx.enter_context(tc.tile_pool(name="ps_z", bufs=2, space="PSUM"))
        ps_h = ctx.enter_context(tc.tile_pool(name="ps_h", bufs=2, space="PSUM"))
        ps_q = ctx.enter_context(tc.tile_pool(name="ps_q", bufs=2, space="PSUM"))

        cb = singles.tile([128, NCOL], F32R, tag="consts")
        nc.sync.dma_start(out=cb[:], in_=consts[:])
        ch = singles.tile([128, NCOLH], F16, tag="constsh")
        nc.sync.dma_start(out=ch[:], in_=constsh[:])

        def w(i):  # f32r weight tile i, lhsT [128, 128]
            return cb[:, i * 128:(i + 1) * 128]

        def wh(i):  # fp16 weight tile i
            return ch[:, i * 128:(i + 1) * 128]

        def tabh(base, st):  # fp16 table [128, nseq, sc] broadcast over seq
            return ch[:, base + st * sc: base + (st + 1) * sc][:, None, :] \
                .to_broadcast([128, nseq, sc])

        # carry state [128, st, plane, b], fp32, zero-init
        carry = carries.tile([128, 2, 2, nseq], F32, tag="carry")
        nc.vector.memset(carry[:], 0.0)

        for sup in range(nsup):
            glo = sup * sc

            # ---- phase A: per sub-chunk, load x^T + Bu matmuls + evac -----
            xt = [[None, None] for _ in range(subs)]
            bu_sb = {}  # (st, plane) -> [128, nseq, sc] fp16
            for st in range(2):
                for pl in range(2):
                    bu_sb[(st, pl)] = bus.tile([128, nseq, sc], F16,
                                               tag=f"bu{st}{pl}",
                                               name=f"bu{st}{pl}")
            for sub in range(subs):
                lo = glo + sub * pc
                for kt in range(2):
                    t = xts.tile([128, nseq, pc], F32R, tag=f"xt{kt}")
                    nc.sync.dma_start(out=t[:], in_=xTv[kt][:, :, lo:lo + pc])
                    xt[sub][kt] = t
                for st in range(2):
                    bu_re = ps_bu.tile([128, nseq, pc], F32, tag="bu")
                    bu_im = ps_bu.tile([128, nseq, pc], F32, tag="bu")
                    for pl, ps in ((0, bu_re), (1, bu_im)):
                        for kt in range(2):
                            wi = _wi_bnre(kt, st) if pl == 0 else _wi_bnim(kt, st)
                            nc.tensor.matmul(ps[:], w(wi), xt[sub][kt][:],
                                             start=(kt == 0), stop=(kt == 1))
                        nc.scalar.activation(
                            bu_sb[(st, pl)][:, :, sub * pc:(sub + 1) * pc],
                            ps[:], ACTF.Copy,
                        )

            # ---- phase B: twiddle + scans + untwiddle (super-chunk, fp16) -
            s_re, s_im, v_re, v_im = [], [], [], []
            for st in range(2):
                cosb, sinb = tabh(CH0, st), tabh(SH0, st)
                bre, bim = bu_sb[(st, 0)], bu_sb[(st, 1)]
                ut_re = uts.tile([128, nseq, sc], F16, tag=f"utre{st}")
                ut_im = uts.tile([128, nseq, sc], F16, tag=f"utim{st}")
                t1 = tmps.tile([128, nseq, sc], F16, tag="twtmp")
                t2 = tmps.tile([128, nseq, sc], F16, tag="twtmp")
                nc.vector.tensor_tensor(t1[:], cosb, bre[:], Alu.mult)
                nc.vector.tensor_tensor(t2[:], sinb, bim[:], Alu.mult)
                nc.vector.tensor_tensor(ut_re[:], t1[:], t2[:], Alu.add)
                t3 = tmps.tile([128, nseq, sc], F16, tag="twtmp")
                t4 = tmps.tile([128, nseq, sc], F16, tag="twtmp")
                nc.vector.tensor_tensor(t3[:], cosb, bim[:], Alu.mult)
                nc.vector.tensor_tensor(t4[:], sinb, bre[:], Alu.mult)
                nc.vector.tensor_tensor(ut_im[:], t3[:], t4[:], Alu.subtract)

                # scans run in place: v overwrites ut
                vr, vi = ut_re, ut_im
                rt2 = cb[:, RT0 + st * sc: RT0 + (st + 1) * sc].bitcast(F32)
                for b in range(nseq):
                    nc.vector.tensor_tensor_scan(
                        vr[:, b, :], rt2, ut_re[:, b, :],
                        carry[:, st, 0, b:b + 1], Alu.mult, Alu.add)
                    nc.vector.tensor_tensor_scan(
                        vi[:, b, :], rt2, ut_im[:, b, :],
                        carry[:, st, 1, b:b + 1], Alu.mult, Alu.add)
                v_re.append(vr)
                v_im.append(vi)

                sr = ss.tile([128, nseq, sc], F16, tag=f"sre{st}")
                si_ = ss.tile([128, nseq, sc], F16, tag=f"sim{st}")
                u1 = tmps.tile([128, nseq, sc], F16, tag="twtmp")
                u2 = tmps.tile([128, nseq, sc], F16, tag="twtmp")
                nc.vector.tensor_tensor(u1[:], cosb, vr[:], Alu.mult)
                nc.vector.tensor_tensor(u2[:], sinb, vi[:], Alu.mult)
                nc.vector.tensor_tensor(sr[:], u1[:], u2[:], Alu.subtract)
                u3 = tmps.tile([128, nseq, sc], F16, tag="twtmp")
                u4 = tmps.tile([128, nseq, sc], F16, tag="twtmp")
                nc.vector.tensor_tensor(u3[:], cosb, vi[:], Alu.mult)
                nc.vector.tensor_tensor(u4[:], sinb, vr[:], Alu.mult)
                nc.vector.tensor_tensor(si_[:], u3[:], u4[:], Alu.add)
                s_re.append(sr)
                s_im.append(si_)

            # ---- carry update: c' = e^{i th sc} * v[:, :, -1] --------------
            carry_new = carries.tile([128, 2, 2, nseq], F32, tag="carry")
            for st in range(2):
                rotc = cb[:, ROT0 + st: ROT0 + st + 1].bitcast(F32)
                rots = cb[:, ROT0 + 2 + st: ROT0 + 3 + st].bitcast(F32)
                vrl = v_re[st][:, :, sc - 1]
                vil = v_im[st][:, :, sc - 1]
                ta = tmps.tile([128, nseq], F32, tag="cartmp")
                nc.vector.tensor_scalar_mul(ta[:], vil, rots)
                nc.vector.scalar_tensor_tensor(
                    carry_new[:, st, 0, :], vrl, rotc, ta[:], Alu.mult,
                    Alu.subtract)
                tb = tmps.tile([128, nseq], F32, tag="cartmp")
                nc.vector.tensor_scalar_mul(tb[:], vrl, rots)
                nc.vector.scalar_tensor_tensor(
                    carry_new[:, st, 1, :], vil, rotc, tb[:], Alu.mult, Alu.add)
            carry = carry_new

            # ---- phase C: per sub-chunk, z / MLP / residual / store --------
            for sub in range(subs):
                lo = glo + sub * pc
                wlo, whi = sub * pc, (sub + 1) * pc

                z_sb = []
                for ot in range(2):
                    zp = ps_z.tile([128, nseq, pc], F32, tag="z")
                    nc.tensor.matmul(zp[:], wh(_hi_cr(0, ot)),
                                     s_re[0][:, :, wlo:whi], start=True, stop=False)
                    nc.tensor.matmul(zp[:], wh(_hi_cr(1, ot)),
                                     s_re[1][:, :, wlo:whi], start=False, stop=False)
                    nc.tensor.matmul(zp[:], wh(_hi_cm(0, ot)),
                                     s_im[0][:, :, wlo:whi], start=False, stop=False)
                    nc.tensor.matmul(zp[:], wh(_hi_cm(1, ot)),
                                     s_im[1][:, :, wlo:whi], start=False, stop=False)
                    nc.tensor.matmul(zp[:], w(_wi_dT(0, ot)), xt[sub][0][:],
                                     start=False, stop=False)
                    nc.tensor.matmul(zp[:], w(_wi_dT(1, ot)), xt[sub][1][:],
                                     start=False, stop=True)
                    zt = zs_p.tile([128, nseq, pc], F32R, tag=f"z{ot}")
                    nc.scalar.activation(zt[:], zp[:], ACTF.Copy)
                    z_sb.append(zt)

                h_sb = []
                for ft in range(8):
                    hp = ps_h.tile([128, nseq, pc], F32, tag="h")
                    nc.tensor.matmul(hp[:], w(_wi_wfc(0, ft)), z_sb[0][:],
                                     start=True, stop=False)
                    nc.tensor.matmul(hp[:], w(_wi_wfc(1, ft)), z_sb[1][:],
                                     start=False, stop=True)
                    ht = hs_p.tile([128, nseq, pc], F32R, tag=f"h{ft}")
                    nc.scalar.activation(
                        ht[:], hp[:], GELU_FUNC,
                        bias=cb[:, BFC0 + ft: BFC0 + ft + 1].bitcast(F32),
                        scale=1.0)
                    h_sb.append(ht)

                for ot in range(2):
                    qp = ps_q.tile([128, nseq, pc], F32, tag="q")
                    for ft in range(8):
                        nc.tensor.matmul(qp[:], w(_wi_wpj(ft, ot)), h_sb[ft][:],
                                         start=(ft == 0), stop=False)
                    nc.tensor.matmul(qp[:], w(_WI_IDENT), xt[sub][ot][:],
                                     start=False, stop=True)
                    ob = obs.tile([128, nseq, pc], F32, tag=f"ob{ot}")
                    nc.scalar.activation(
                        ob[:], qp[:], ACTF.Identity,
                        bias=cb[:, BPJ0 + ot: BPJ0 + ot + 1].bitcast(F32),
                        scale=1.0)
                    nc.sync.dma_start(out=outTv[ot][:, :, lo:lo + pc], in_=ob[:])
    return nc


def pack_consts(nu_log, theta_log, gamma_log, B_re, B_im, C_re, C_im, D,
                W_fc, b_fc, W_proj, b_proj, sc=SC):
    """Assemble the f32r and fp16 consts blobs (tables in float64)."""
    f8 = np.float64
    nu = np.exp(np.asarray(nu_log, f8))
    r = np.exp(-nu)
    theta = np.exp(np.asarray(theta_log, f8))
    gamma = np.exp(np.asarray(gamma_log, f8))
    Bn_re = np.asarray(B_re, f8) * gamma[:, None]
    Bn_im = np.asarray(B_im, f8) * gamma[:, None]
    C_re = np.asarray(C_re, f8)
    C_im = np.asarray(C_im, f8)
    D = np.asarray(D, f8)
    W_fc = np.asarray(W_fc, f8)
    W_proj = np.asarray(W_proj, f8)

    cb = np.zeros((128, NCOL), np.float32)
    ch = np.zeros((128, NCOLH), np.float16)

    def put(i, m):
        cb[:, i * 128:(i + 1) * 128] = np.asarray(m, np.float32)

    def puth(i, m):
        ch[:, i * 128:(i + 1) * 128] = np.asarray(m, np.float16)

    for kt in range(2):
        for st in range(2):
            put(_wi_bnre(kt, st),
                Bn_re[st * 128:(st + 1) * 128, kt * 128:(kt + 1) * 128].T)
            put(_wi_bnim(kt, st),
                Bn_im[st * 128:(st + 1) * 128, kt * 128:(kt + 1) * 128].T)
    for st in range(2):
        for ot in range(2):
            puth(_hi_cr(st, ot),
                 C_re[ot * 128:(ot + 1) * 128, st * 128:(st + 1) * 128].T)
            puth(_hi_cm(st, ot),
                 -C_im[ot * 128:(ot + 1) * 128, st * 128:(st + 1) * 128].T)
    for kt in range(2):
        for ot in range(2):
            put(_wi_dT(kt, ot),
                D[ot * 128:(ot + 1) * 128, kt * 128:(kt + 1) * 128].T)
    for kt in range(2):
        for ft in range(8):
            put(_wi_wfc(kt, ft),
                W_fc[kt * 128:(kt + 1) * 128, ft * 128:(ft + 1) * 128])
    for ft in range(8):
        for ot in range(2):
            put(_wi_wpj(ft, ot),
                W_proj[ft * 128:(ft + 1) * 128, ot * 128:(ot + 1) * 128])
    put(_WI_IDENT, np.eye(128))

    tau = np.arange(sc, dtype=f8)
    for st in range(2):
        th = theta[st * 128:(st + 1) * 128]
        ang = th[:, None] * tau[None, :]
        ch[:, CH0 + st * sc: CH0 + (st + 1) * sc] = np.cos(ang)
        ch[:, SH0 + st * sc: SH0 + (st + 1) * sc] = np.sin(ang)
        cb[:, RT0 + st * sc: RT0 + (st + 1) * sc] = r[st * 128:(st + 1) * 128, None]
        cb[:, ROT0 + st] = np.cos(th * sc)
        cb[:, ROT0 + 2 + st] = np.sin(th * sc)
    for ft in range(8):
        cb[:, BFC0 + ft] = np.asarray(b_fc, np.float32)[ft * 128:(ft + 1) * 128]
    for ot in range(2):
        cb[:, BPJ0 + ot] = np.asarray(b_proj, np.float32)[ot * 128:(ot + 1) * 128]
    return cb, ch


_NC_CACHE = {}
LAST_RUN_INFO = {}


def kernel(x, nu_log, theta_log, gamma_log, B_re, B_im, C_re, C_im, D,
           W_fc, b_fc, W_proj, b_proj):
    x = np.asarray(x, np.float32)
    assert x.shape == (BATCH, SEQLEN, DM)

    key = (SEQLEN, NSEQ, PC, SC)
    if key not in _NC_CACHE:
        _NC_CACHE[key] = build_nc(SEQLEN, NSEQ, PC, SC)
    nc = _NC_CACHE[key]

    cb, ch = pack_consts(nu_log, theta_log, gamma_log, B_re, B_im, C_re, C_im,
                         D, W_fc, b_fc, W_proj, b_proj, SC)

    in_maps = []
    for c in range(NCORES):
        xc = x[c * NSEQ:(c + 1) * NSEQ]                      # (nseq, L, D)
        xT = np.ascontiguousarray(
            xc.transpose(2, 0, 1).reshape(2, 128, NSEQ * SEQLEN)
        )
        in_maps.append({"xT": xT, "consts": cb, "constsh": ch})

    res = run_bass_kernel_spmd(nc, in_maps, core_ids=list(range(NCORES)))
    LAST_RUN_INFO.clear()
    LAST_RUN_INFO.update(
        exec_time_ns=res.exec_time_ns,
        mean_exec_time_ns=res.mean_exec_time_ns,
        trace=res.instructions_and_trace[1] if res.instructions_and_trace else None,
        profile_json=res.profile_json,
    )

    out = np.empty((BATCH, SEQLEN, DM), np.float32)
    for c in range(NCORES):
        oT = res.results[c]["outT"]                          # (2, 128, ntok)
        out[c * NSEQ:(c + 1) * NSEQ] = (
            oT.reshape(DM, NSEQ, SEQLEN).transpose(1, 2, 0)
        )
    return out
